# revision 42
# baseline (speedup 1.0000x reference)
"""Sparse (log-mask) attention with entmax15 — Trainium2 Bass kernel, v2.

Sharding: 8 cores, core c handles head h=c for both batch rows.  Each core
computes its head's partial output (att @ V @ Wp); host sums the 8 partials
and adds b_proj + b_v @ w_proj (b_v folds exactly: entmax rows sum to 1).

Pipeline (all fp16 storage, fp32 PSUM/accum):
  - x is host-cast to fp16; xT via DMA-xbar transpose straight from DRAM.
  - conv q/k: 5 shifted fp16 matmuls per 512-chunk; ACT evicts with bias.
  - vT = wv @ xT; VP = (v @ w_proj) precomputed per batch (proj folded into
    the AV matmul rhs), stored in chunked-natural layout.
  - scores: per 128-row tile, PSUM starts with the mask (identity-matmul of
    the packed fp16 mask, NEG=-30000) and QK accumulates on top.
  - entmax15 tau solve per row: tau = tau0 + sigma, tau0 = diag-block max - 2
    (read from PSUM).  ACT Relu-evict writes y0 = relu(S - tau0) fp16 and
    accumulates R1(0); a Square pass gives R2(0).  Then 2 Newton steps on
    g = sqrt(R2) - 2 with exact R2 re-measured at each snapped sigma via
    A1/A2 identities (R1 = A1 - W*sigma, R2 = A2 - 2*sigma*A1 + W*sigma^2;
    sigma snapped to fp16 so clipped lanes contribute exactly), then a final
    unsnapped Newton step.  Final pass: yf = relu(y0 - sigma3) (DVE 4x),
    att = yf^2 with row-sum accum (ACT Square), normalization folded into
    the AV-PSUM eviction scale (1/rowsum).
  - att transposed via DMA-xbar; AV matmul against VP chunks -> po[b,s,:].
Sweeps over the two batches are interleaved (A/B) so DVE/ACT/PE overlap.
"""

import numpy as np
import ml_dtypes

B = 2
S = 2048
D = 128
H = 8
QL = 5
NEG = -30000.0
NTILE = S // 128  # 16 row tiles
# fraction of tiles whose square pass runs on ACT (rest on DVE ttr), by phase
FR_SQ0 = 1.0
FR_SW = 0.9
FR_FIN = 0.8
FR_EV = 0.3  # fraction of tiles evicted by ACT Relu (rest DVE max from PSUM)
PESD0 = 9   # tiles >= this use PE self-dot for the sweep square
PESD0_SQ0 = 16  # tiles >= this defer SQ0 to the PE self-dot (16 = off)


def _on_act(i, frac):
    # strided split (used for evict balance)
    return ((i * 5) % 16) < 16 * frac


def _on_act_prefix(i, wfrac):
    # ACT takes the leading tiles covering ~wfrac of total masked width, so
    # its queue drains before the batch-level stats (sqrt) needs it
    return OFF[i + 1] <= wfrac * TOTW

_CACHE = {}

# packed mask column offsets: tile i occupies [OFF[i], OFF[i] + (i+1)*128)
OFF = [0]
for _i in range(NTILE):
    OFF.append(OFF[-1] + (_i + 1) * 128)
TOTW = OFF[-1]  # 17408


def _build_program(repeat=1):
    import concourse.bass as bass
    import concourse.mybir as mybir
    import concourse.tile as tile
    from concourse import bacc
    from concourse.bass import ts
    from concourse.masks import make_identity

    f32 = mybir.dt.float32
    f16 = mybir.dt.float16
    AF = mybir.ActivationFunctionType
    OP = mybir.AluOpType

    nc = bacc.Bacc("TRN2", target_bir_lowering=False, debug=False,
                   enable_asserts=False)

    x_d = nc.dram_tensor("x", [B, S, D], f16, kind="ExternalInput").ap()
    wqk_d = nc.dram_tensor("wqk", [2 * QL * D, D], f16, kind="ExternalInput").ap()
    bq_d = nc.dram_tensor("bq", [D, 1], f32, kind="ExternalInput").ap()
    bk_d = nc.dram_tensor("bk", [D, 1], f32, kind="ExternalInput").ap()
    wv_d = nc.dram_tensor("wv", [D, D], f16, kind="ExternalInput").ap()
    wp_d = nc.dram_tensor("wp", [D, D], f16, kind="ExternalInput").ap()
    bv_d = nc.dram_tensor("bv", [D, 1], f32, kind="ExternalInput").ap()
    nm_d = nc.dram_tensor("nmask", [128, TOTW], f16, kind="ExternalInput").ap()
    po_d = nc.dram_tensor("po", [B, S, D], f16, kind="ExternalOutput").ap()

    with tile.TileContext(nc) as tc:
        for _rep in range(repeat):
            _body(nc, tc, tile, mybir, f32, f16, AF, OP, ts, make_identity,
                  x_d, wqk_d, bq_d, bk_d, wv_d, wp_d, bv_d, nm_d, po_d)
    nc.compile()
    return nc


def _body(nc, tc, tile, mybir, f32, f16, AF, OP, ts, make_identity,
          x_d, wqk_d, bq_d, bk_d, wv_d, wp_d, bv_d, nm_d, po_d):
    from contextlib import ExitStack

    AX = mybir.AxisListType.X

    ctx = ExitStack()
    with ctx:
        cpool = ctx.enter_context(tc.tile_pool(name="consts", bufs=1))
        vpp = ctx.enter_context(tc.tile_pool(name="vp", bufs=2))
        y0p = ctx.enter_context(tc.tile_pool(name="y0", bufs=2))
        mscr = ctx.enter_context(tc.tile_pool(name="mscr", bufs=7))
        avp = ctx.enter_context(tc.tile_pool(name="avs", bufs=2))
        stp = ctx.enter_context(tc.tile_pool(name="st", bufs=48))
        ictx = ExitStack()  # inner scope: freed after the scores phase
        ps_sc = ictx.enter_context(tc.tile_pool(name="pssc", bufs=3, space="PSUM"))
        ps_qk = ictx.enter_context(tc.tile_pool(name="psqk", bufs=2, space="PSUM"))
        xtp = ictx.enter_context(tc.tile_pool(name="xt", bufs=2))
        qkp = ictx.enter_context(tc.tile_pool(name="qk", bufs=4))
        vtp = ictx.enter_context(tc.tile_pool(name="vt", bufs=2))
        nmp = ictx.enter_context(tc.tile_pool(name="nmsk", bufs=1))

        ident = cpool.tile([128, 128], f16, tag="ident")
        make_identity(nc, ident)

        wq_sb = cpool.tile([128, QL * 128], f16, tag="wq")
        wk_sb = cpool.tile([128, QL * 128], f16, tag="wk")
        nc.sync.dma_start(
            wq_sb[:], wqk_d[0:QL * D, :].rearrange("(t p) f -> p t f", p=128))
        nc.sync.dma_start(
            wk_sb[:], wqk_d[QL * D:2 * QL * D, :].rearrange(
                "(t p) f -> p t f", p=128))
        wv_sb = cpool.tile([128, 128], f16, tag="wv")
        wp_sb = cpool.tile([128, 128], f16, tag="wp")
        nc.sync.dma_start(wv_sb[:], wv_d[:])
        nc.sync.dma_start(wp_sb[:], wp_d[:])
        bq_sb = cpool.tile([128, 1], f32, tag="bq")
        bk_sb = cpool.tile([128, 1], f32, tag="bk")
        bv_sb = cpool.tile([128, 1], f32, tag="bv")
        nc.sync.dma_start(bq_sb[:], bq_d[:])
        nc.sync.dma_start(bk_sb[:], bk_d[:])
        nc.sync.dma_start(bv_sb[:], bv_d[:])

        # per-tile width constants [128, NTILE] (value (i+1)*128 in col i)
        wv_const = cpool.tile([128, NTILE], f32, tag="wconst")
        for i in range(NTILE):
            nc.vector.memset(wv_const[:, i:i + 1], float((i + 1) * 128))

        PAD = QL - 1

        # ---------------- setup per batch: xT, q, k, vT, VP ----------------
        xT, qT, kT, vp_nat = [], [], [], []
        for b in range(B):
            xt = xtp.tile([128, S + PAD], f16, tag="xt")
            x0 = xtp.tile([128, S], f16, tag="x0")
            nc.vector.memset(xt[:, 0:PAD], 0.0)
            for c in range(4):
                nc.sync.dma_start_transpose(
                    x0[:, c * 512:(c + 1) * 512],
                    x_d[b, c * 512:(c + 1) * 512, :])
                nc.vector.tensor_copy(xt[:, PAD + c * 512: PAD + (c + 1) * 512],
                                      x0[:, c * 512:(c + 1) * 512])
            xT.append(xt)

        # packed mask: [128, TOTW] fp16, loaded once, read by both batches
        nm_sb = nmp.tile([128, TOTW], f16, tag="nm")
        NMC = 4
        for c in range(NMC):
            w0 = (TOTW // NMC) * c
            w1 = TOTW if c == NMC - 1 else (TOTW // NMC) * (c + 1)
            nc.sync.dma_start(nm_sb[:, w0:w1], nm_d[:, w0:w1])

        for b in range(B):
            qt = qkp.tile([128, S], f16, tag="qT")
            kt = qkp.tile([128, S], f16, tag="kT")
            vt = vtp.tile([128, S], f16, tag="vT")
            for n in range(S // 512):
                for (dst, w_sb, b_sb) in ((qt, wq_sb, bq_sb), (kt, wk_sb, bk_sb)):
                    pq = ps_qk.tile([128, 512], f32, tag="psqk")
                    for t in range(QL):
                        sh = QL - 1 - t
                        nc.tensor.matmul(
                            pq[:], w_sb[:, ts(t, 128)],
                            xT[b][:, PAD + n * 512 - sh: PAD + n * 512 - sh + 512],
                            start=(t == 0), stop=(t == QL - 1))
                    nc.scalar.activation(dst[:, ts(n, 512)], pq[:],
                                         AF.Identity, bias=b_sb[:])
                pv = ps_qk.tile([128, 512], f32, tag="psqk")
                nc.tensor.matmul(pv[:], wv_sb[:],
                                 xT[b][:, PAD + n * 512: PAD + (n + 1) * 512],
                                 start=True, stop=True)
                nc.scalar.activation(vt[:, ts(n, 512)], pv[:], AF.Identity,
                                     bias=bv_sb[:])
            qT.append(qt)
            kT.append(kt)
            # VP = v @ w_proj in chunked-natural layout [128, NTILE*128]
            vp = vpp.tile([128, S], f16, tag="vp")
            for j0 in range(0, NTILE, 4):
                pw = ps_qk.tile([128, 512], f32, tag="psqk")
                for j in range(j0, j0 + 4):
                    nc.tensor.matmul(pw[:, ts(j - j0, 128)], vt[:, ts(j, 128)],
                                     wp_sb[:], start=True, stop=True)
                nc.scalar.activation(vp[:, j0 * 128: j0 * 128 + 512], pw[:],
                                     AF.Identity)
            vp_nat.append(vp)

        # ---------------- stats tiles per batch ----------------
        # cols = tile index
        _stat_n = [0]

        def stat():
            out = []
            for _b in range(B):
                _stat_n[0] += 1
                out.append(stp.tile([128, NTILE], f32, tag="st",
                                    name=f"st{_stat_n[0]}"))
            return out

        tau0 = stat()      # fp32 raw tau0 per tile col
        ntau0 = stat()
        r1c = stat()       # running R1 at current sigma
        r2c = stat()       # running R2 at current sigma
        sig = stat()       # current sigma (snapped fp32)
        sig16 = [stp.tile([128, NTILE], f16, tag="st16", name=f"st16_{_b}")
                 for _b in range(B)]
        acc_a = stat()     # evict accum chunk A
        acc_b = stat()     # evict accum chunk B
        a1t = stat()       # sweep A1 accums
        a2t = stat()       # sweep A2 accums
        rs = stat()        # final rowsum
        invr = stat()
        sig3 = stat()      # final sigma (snapped)
        nsig3 = stat()
        nsig = stat()      # -sig
        sigp = stat()      # previous sigma (for trapezoid RS)
        r1p = stat()       # previous R1

        y0_all = [y0p.tile([128, TOTW], f16, tag="y0all", name=f"y0all{_b}")
                  for _b in range(B)]

        def y0_t(b, i):
            return y0_all[b][:, OFF[i]:OFF[i] + (i + 1) * 128]

        # ---------------- phase 1: scores + relu-evict + SQ0 ----------------
        def scores_tile(b, i):
            W = (i + 1) * 128
            nch = 1 if W <= 1024 else 2
            # chunk 1 (diag-containing): [W - cw1, W); chunk 0: [0, W - cw1)
            cw1 = min(W, 1024)
            c1_0 = W - cw1
            y0 = y0_t(b, i)
            ps1 = ps_sc.tile([128, 1024], f32, tag="pssc")
            for sub in range(0, cw1, 512):
                sw = min(512, cw1 - sub)
                nc.tensor.matmul(ps1[:, sub:sub + sw], ident[:],
                                 nm_sb[:, OFF[i] + c1_0 + sub: OFF[i] + c1_0 + sub + sw],
                                 start=True, stop=False)
                nc.tensor.matmul(ps1[:, sub:sub + sw], qT[b][:, ts(i, 128)],
                                 kT[b][:, c1_0 + sub: c1_0 + sub + sw],
                                 start=False, stop=True)
            # diag block max -> tau0 = dmax - 2
            dg0 = cw1 - 128
            nc.vector.tensor_reduce(tau0[b][:, i:i + 1], ps1[:, dg0:dg0 + 128],
                                    AX, OP.max)
            nc.vector.tensor_scalar_add(tau0[b][:, i:i + 1], tau0[b][:, i:i + 1],
                                        -2.0)
            ev_act = _on_act(i + 3, FR_EV)
            nc.vector.tensor_scalar_mul(ntau0[b][:, i:i + 1],
                                        tau0[b][:, i:i + 1], -1.0)
            if ev_act:
                # theta0 = 0: z = relu(S - tau0); accum = R1 directly
                nc.vector.memset(sig[b][:, i:i + 1], 0.0)
                nc.scalar.activation(y0[:, c1_0:W], ps1[:, 0:cw1], AF.Relu,
                                     bias=ntau0[b][:, i:i + 1],
                                     accum_out=acc_a[b][:, i:i + 1])
            else:
                # theta0 = snapped tau0: z = max(S, tau0s); accum = A1
                nc.vector.tensor_copy(sig16[b][:, i:i + 1], tau0[b][:, i:i + 1])
                nc.vector.tensor_copy(sig[b][:, i:i + 1], sig16[b][:, i:i + 1])
                nc.vector.tensor_scalar(out=y0[:, c1_0:W], in0=ps1[:, 0:cw1],
                                        scalar1=sig[b][:, i:i + 1],
                                        scalar2=None, op0=OP.max, op1=OP.add,
                                        accum_out=acc_a[b][:, i:i + 1])
            if nch == 2:
                cw0 = W - 1024
                ps0 = ps_sc.tile([128, 1024], f32, tag="pssc")
                for sub in range(0, cw0, 512):
                    sw = min(512, cw0 - sub)
                    nc.tensor.matmul(ps0[:, sub:sub + sw], ident[:],
                                     nm_sb[:, OFF[i] + sub: OFF[i] + sub + sw],
                                     start=True, stop=False)
                    nc.tensor.matmul(ps0[:, sub:sub + sw], qT[b][:, ts(i, 128)],
                                     kT[b][:, sub: sub + sw],
                                     start=False, stop=True)
                if ev_act:
                    nc.scalar.activation(y0[:, 0:cw0], ps0[:, 0:cw0], AF.Relu,
                                         bias=ntau0[b][:, i:i + 1],
                                         accum_out=acc_b[b][:, i:i + 1])
                else:
                    nc.vector.tensor_scalar(out=y0[:, 0:cw0], in0=ps0[:, 0:cw0],
                                            scalar1=sig[b][:, i:i + 1],
                                            scalar2=None, op0=OP.max,
                                            op1=OP.add,
                                            accum_out=acc_b[b][:, i:i + 1])
            else:
                nc.vector.memset(acc_b[b][:, i:i + 1], 0.0)
            # SQ0: R2(0) via square accum on y0 (requires ACT-evicted tile
            # for the DVE path: y0 = relu there, so y0^2 is cancellation-free)
            if i >= PESD0_SQ0:
                pass  # deferred to sq0_pe (PE self-dot after pool switch)
            elif _on_act_prefix(i, FR_SQ0) or not ev_act:
                if not ev_act:
                    nth = stp.tile([128, 1], f32, tag="nth", name=f"nth{b}_{i}")
                    nc.vector.tensor_scalar_mul(nth[:], sig[b][:, i:i + 1],
                                                -1.0)
                    bias0 = nth[:]
                else:
                    bias0 = 0.0
                scr = mscr.tile([128, S], f16, tag="mscr")
                nc.scalar.activation(scr[:, 0:W], y0[:, 0:W], AF.Square,
                                     bias=bias0,
                                     accum_out=a2t[b][:, i:i + 1])
                del scr
            else:
                sq = mscr.tile([128, S], f16, tag="mscr")
                nc.vector.tensor_tensor(sq[:, 0:W], y0[:, 0:W], y0[:, 0:W],
                                        OP.mult)
                nc.vector.tensor_reduce(a2t[b][:, i:i + 1], sq[:, 0:W], AX,
                                        OP.add)

        def scores_finish(b):
            nc.vector.tensor_tensor(a1t[b][:], acc_a[b][:], acc_b[b][:], OP.add)

        # deferred SQ0 for wide tiles: R2(theta0) via PE self-dot of
        # yr0 = z - theta0 (exact zeros; theta0 snapped for DVE-evicted tiles)
        def sq0_pe(b, atT):
            nth0 = stp.tile([128, NTILE], f32, tag="st", name=f"nth0_{b}")
            nc.vector.tensor_scalar_mul(nth0[:], sig[b][:], -1.0)
            for i in range(PESD0_SQ0, NTILE):
                W = (i + 1) * 128
                yr = mscr.tile([128, S], f16, tag="mscr")
                nc.vector.tensor_scalar(out=yr[:, 0:W], in0=y0_t(b, i),
                                        scalar1=nth0[:, i:i + 1],
                                        scalar2=None, op0=OP.add)
                c0 = OFF[i] // 128
                nc.sync.dma_start_transpose(
                    atT[b][:, c0:c0 + i + 1, :], yr[:, 0:W])
                psd = ps_av.tile([128, 128], f32, tag="psav")
                for j in range(i + 1):
                    nc.tensor.matmul(psd[:], atT[b][:, c0 + j, :],
                                     atT[b][:, c0 + j, :],
                                     start=(j == 0), stop=(j == i))
                dtmp = avp.tile([128, 128], f32, tag="avs")
                nc.vector.tensor_tensor(dtmp[:], psd[:], ident[:], OP.mult)
                nc.vector.tensor_reduce(a2t[b][:, i:i + 1], dtmp[:],
                                        AX, OP.add)

        # ---------------- newton step (stats only) ----------------
        def newton_sigma(b, out_sig, out_nsig=None, snap=True):
            """sig' = sig + max((sqrt(R2)-2)*sqrt(R2)/R1, 0), optionally
            fp16-snapped, into out_sig."""
            t0 = stp.tile([128, NTILE], f32, tag="st")
            sq = stp.tile([128, NTILE], f32, tag="st")
            nc.vector.tensor_scalar_max(t0[:], r2c[b][:], 0.0)
            nc.scalar.activation(sq[:], t0[:], AF.Sqrt)
            g = stp.tile([128, NTILE], f32, tag="st")
            nc.vector.tensor_scalar_add(g[:], sq[:], -2.0)
            nc.vector.tensor_tensor(g[:], g[:], sq[:], OP.mult)
            rc = stp.tile([128, NTILE], f32, tag="st")
            nc.vector.tensor_scalar_max(rc[:], r1c[b][:], 1e-6)
            nc.vector.reciprocal(rc[:], rc[:])
            nc.vector.tensor_tensor(g[:], g[:], rc[:], OP.mult)
            nc.vector.tensor_scalar_max(g[:], g[:], 0.0)
            nc.vector.tensor_tensor(out_sig[:], sig[b][:], g[:], OP.add)
            if snap:
                nc.vector.tensor_copy(sig16[b][:], out_sig[:])
                nc.vector.tensor_copy(out_sig[:], sig16[b][:])
            if out_nsig is not None:
                nc.vector.tensor_scalar_mul(out_nsig[:], out_sig[:], -1.0)

        # measured sweep passes at snapped sigma (already in sig[b]).
        # Widest tiles (>= PESD0) compute sum(m^2) on the idle PE via a
        # self-dot of the xbar-transposed m; sweep_stats converts A2 -> R2
        # with the sigma identity for those columns.
        def sweep_passes(b, atT):
            for i in range(NTILE):
                W = (i + 1) * 128
                y0 = y0_t(b, i)
                m = mscr.tile([128, S], f16, tag="mscr")
                nc.vector.tensor_scalar(out=m[:, 0:W], in0=y0[:, 0:W],
                                        scalar1=sig[b][:, i:i + 1],
                                        scalar2=None, op0=OP.max, op1=OP.add,
                                        accum_out=a1t[b][:, i:i + 1])
                if i >= PESD0:
                    yr = mscr.tile([128, S], f16, tag="mscr")
                    nc.vector.tensor_scalar(out=yr[:, 0:W], in0=m[:, 0:W],
                                            scalar1=nsig[b][:, i:i + 1],
                                            scalar2=None, op0=OP.add)
                    c0 = OFF[i] // 128
                    nc.sync.dma_start_transpose(
                        atT[b][:, c0:c0 + i + 1, :], yr[:, 0:W])
                    psd = ps_av.tile([128, 128], f32, tag="psav")
                    for j in range(i + 1):
                        nc.tensor.matmul(psd[:], atT[b][:, c0 + j, :],
                                         atT[b][:, c0 + j, :],
                                         start=(j == 0), stop=(j == i))
                    dtmp = avp.tile([128, 128], f32, tag="avs")
                    nc.vector.tensor_tensor(dtmp[:], psd[:], ident[:], OP.mult)
                    nc.vector.tensor_reduce(a2t[b][:, i:i + 1], dtmp[:],
                                            AX, OP.add)
                elif _on_act_prefix(i, FR_SW):
                    # R2 direct: Square(m - sigma) with exact zeros at clip
                    nc.scalar.activation(m[:, 0:W], m[:, 0:W], AF.Square,
                                         bias=nsig[b][:, i:i + 1],
                                         accum_out=a2t[b][:, i:i + 1])
                else:
                    yr = mscr.tile([128, S], f16, tag="mscr")
                    nc.vector.tensor_scalar(out=yr[:, 0:W], in0=m[:, 0:W],
                                            scalar1=nsig[b][:, i:i + 1],
                                            scalar2=None, op0=OP.add)
                    nc.vector.tensor_tensor(yr[:, 0:W], yr[:, 0:W],
                                            yr[:, 0:W], OP.mult)
                    nc.vector.tensor_reduce(a2t[b][:, i:i + 1], yr[:, 0:W],
                                            AX, OP.add)

        # R1 = A1 - W*sig ; R2 = A2 directly, except self-dot columns where
        # R2 = A2 - sig*(2*A1 - sig*W) = A2 - sig*(A1 + R1)
        def sweep_stats(b):
            t0 = stp.tile([128, NTILE], f32, tag="st")
            nc.vector.tensor_tensor(t0[:], sig[b][:], wv_const[:], OP.mult)
            nc.vector.tensor_tensor(r1c[b][:], a1t[b][:], t0[:], OP.subtract)
            nc.vector.tensor_copy(r2c[b][:], a2t[b][:])

        # final phase: snapped sigma3; per tile: m-pass (R1_3), then either
        # ACT Square(m,-s3) with measured RS, or DVE yf+square with
        # trapezoid RS.  att written back over y0, then invr + transpose.
        def final_passes(b, atT):
            nc.vector.tensor_copy(sigp[b][:], sig[b][:])
            nc.vector.tensor_copy(r1p[b][:], r1c[b][:])
            newton_sigma(b, sig3[b], out_nsig=nsig3[b], snap=True)
            for i in range(NTILE):
                W = (i + 1) * 128
                y0 = y0_t(b, i)
                m = mscr.tile([128, S], f16, tag="mscr")
                nc.vector.tensor_scalar(out=m[:, 0:W], in0=y0[:, 0:W],
                                        scalar1=sig3[b][:, i:i + 1],
                                        scalar2=None, op0=OP.max, op1=OP.add,
                                        accum_out=a1t[b][:, i:i + 1])
                if _on_act_prefix(i, FR_FIN):
                    nc.scalar.activation(y0[:, 0:W], m[:, 0:W], AF.Square,
                                         bias=nsig3[b][:, i:i + 1],
                                         accum_out=rs[b][:, i:i + 1])
                else:
                    yf = mscr.tile([128, S], f16, tag="mscr")
                    nc.vector.tensor_scalar(out=yf[:, 0:W], in0=m[:, 0:W],
                                            scalar1=nsig3[b][:, i:i + 1],
                                            scalar2=None, op0=OP.add)
                    nc.vector.tensor_tensor(y0[:, 0:W], yf[:, 0:W],
                                            yf[:, 0:W], OP.mult)
                c0 = OFF[i] // 128
                nc.sync.dma_start_transpose(
                    atT[b][:, c0:c0 + i + 1, :], y0_t(b, i))
            # trapezoid RS for the DVE-path tiles:
            # R1_3 = A1 - W*s3 ; RS = R2_2 - (R1_2 + R1_3)*(s3 - s2)
            t0 = stp.tile([128, NTILE], f32, tag="st")
            nc.vector.tensor_tensor(t0[:], sig3[b][:], wv_const[:], OP.mult)
            t1 = stp.tile([128, NTILE], f32, tag="st")
            nc.vector.tensor_tensor(t1[:], a1t[b][:], t0[:], OP.subtract)
            nc.vector.tensor_tensor(t1[:], t1[:], r1p[b][:], OP.add)
            t2 = stp.tile([128, NTILE], f32, tag="st")
            nc.vector.tensor_tensor(t2[:], sig3[b][:], sigp[b][:], OP.subtract)
            nc.vector.tensor_tensor(t1[:], t1[:], t2[:], OP.mult)
            t3 = stp.tile([128, NTILE], f32, tag="st")
            nc.vector.tensor_tensor(t3[:], r2c[b][:], t1[:], OP.subtract)
            for i in range(NTILE):
                if not _on_act_prefix(i, FR_FIN):
                    nc.vector.tensor_copy(rs[b][:, i:i + 1], t3[:, i:i + 1])
                nc.vector.tensor_scalar_max(invr[b][:, i:i + 1],
                                            rs[b][:, i:i + 1], 1e-30)
                nc.vector.reciprocal(invr[b][:, i:i + 1],
                                     invr[b][:, i:i + 1])

        def av_all(b, atT, po_buf):
            for i in range(NTILE):
                nchunk = i + 1
                c0 = OFF[i] // 128
                pav = ps_av.tile([128, 128], f32, tag="psav")
                for j in range(nchunk):
                    nc.tensor.matmul(pav[:], atT[b][:, c0 + j, :],
                                     vp_nat[b][:, ts(j, 128)],
                                     start=(j == 0), stop=(j == nchunk - 1))
                nc.scalar.activation(po_buf[:, ts(i, 128)], pav[:], AF.Copy,
                                     scale=invr[b][:, i:i + 1])
                if i % 4 == 3:
                    nc.gpsimd.dma_start(
                        po_d[b, i * 128 - 384: i * 128 + 128, :].rearrange(
                            "(t p) d -> p t d", p=128),
                        po_buf[:, i * 128 - 384: i * 128 + 128])

        # ---------------- orchestration: A/B interleave ----------------
        for b in range(B):
            for i in range(NTILE):
                scores_tile(b, i)
            scores_finish(b)
        ictx.close()  # free xT/qk/vT/mask SBUF + score PSUM
        ps_av = ctx.enter_context(tc.tile_pool(name="psav", bufs=4, space="PSUM"))
        atp = ctx.enter_context(tc.tile_pool(name="attT", bufs=2))
        atT = [atp.tile([128, TOTW // 128, 128], f16, tag="atT",
                        name=f"atT{_b}") for _b in range(B)]
        for b in range(B):
            sq0_pe(b, atT)
        for b in range(B):
            sweep_stats(b)
            newton_sigma(b, sig[b], out_nsig=nsig[b], snap=True)
        for b in range(B):
            sweep_passes(b, atT)
        for b in range(B):
            sweep_stats(b)
            newton_sigma(b, sig[b], out_nsig=nsig[b], snap=True)
        for b in range(B):
            sweep_passes(b, atT)
        for b in range(B):
            sweep_stats(b)
        for b in range(B):
            final_passes(b, atT)
        pobp = ctx.enter_context(tc.tile_pool(name="pob", bufs=2))
        for b in range(B):
            po_buf = pobp.tile([128, S], f16, tag="pob", name=f"pob{b}")
            av_all(b, atT, po_buf)


def _get_program():
    if "nc" not in _CACHE:
        _CACHE["nc"] = _build_program()
    return _CACHE["nc"]


def _pack_mask(mask2d):
    """[S,S] 0/1 mask -> packed [128, TOTW] fp16 additive mask."""
    nm = (1.0 - mask2d) * NEG
    out = np.zeros((128, TOTW), np.float32)
    for i in range(NTILE):
        W = (i + 1) * 128
        out[:, OFF[i]:OFF[i] + W] = nm[i * 128:(i + 1) * 128, 0:W]
    return out.astype(np.float16)


def _make_in_maps(x, mask, w_qk, b_qk, w_v, b_v, w_proj):
    x = np.asarray(x, np.float32)
    mask2d = np.asarray(mask, np.float32).reshape(S, S)
    w_qk = np.asarray(w_qk, np.float32)
    b_qk = np.asarray(b_qk, np.float32)
    w_v = np.asarray(w_v, np.float32)
    b_v = np.asarray(b_v, np.float32)
    w_proj = np.asarray(w_proj, np.float32)
    scale = np.float32(1.0 / np.sqrt(D))
    nmask = _pack_mask(mask2d)
    x16 = x.astype(np.float16)
    in_maps = []
    for c in range(H):
        qs = slice(c * D, (c + 1) * D)
        ks = slice(H * D + c * D, H * D + (c + 1) * D)
        wq = np.ascontiguousarray(
            np.transpose(w_qk[qs], (2, 1, 0))) * scale      # [QL, d_in, f]
        wk = np.ascontiguousarray(np.transpose(w_qk[ks], (2, 1, 0)))
        wqk = np.concatenate([wq.reshape(QL * D, D),
                              wk.reshape(QL * D, D)], axis=0)
        in_maps.append({
            "x": x16,
            "wqk": wqk.astype(np.float16),
            "bq": (b_qk[qs] * scale).reshape(D, 1).astype(np.float32),
            "bk": b_qk[ks].reshape(D, 1).astype(np.float32),
            "wv": np.ascontiguousarray(w_v[:, qs]).astype(np.float16),
            "wp": np.ascontiguousarray(w_proj[qs]).astype(np.float16),
            "bv": b_v[qs].reshape(D, 1).astype(np.float32),
            "nmask": nmask,
        })
    return in_maps


def kernel(x, mask, w_qk, b_qk, w_v, b_v, w_proj, b_proj, **_):
    from concourse import bass_utils

    nc = _get_program()
    in_maps = _make_in_maps(x, mask, w_qk, b_qk, w_v, b_v, w_proj)
    res = bass_utils.run_bass_kernel_spmd(nc, in_maps, core_ids=list(range(H)))
    acc = np.zeros((B, S, D), np.float64)
    for r in res.results:
        acc += r["po"].astype(np.float64)
    b_eff = (np.asarray(b_proj, np.float64)
             + np.asarray(b_v, np.float64) @ np.asarray(w_proj, np.float64))
    out = (acc + b_eff[None, None, :]).astype(np.float32)
    return out
